# revision 9
# baseline (speedup 1.0000x reference)
"""MultiHeadCDSA Trainium2 kernel (8 NeuronCores, SPMD, no collectives).

Sharding: core c handles batch b = c//2 and row-half r = c%2 (512 of 1024
query rows).  K/V-side work (k/v/feature projections, attn_feature) is
computed redundantly by the two cores of a batch pair; q-side work
(scores, softmax, attn@V chain, fc, layernorm) is split by rows.

All matmul operands are laid out so the contraction dim sits on SBUF
partitions; the host pre-transposes activations/weights so the device
never transposes inputs.  Softmax skips max-subtraction (scores are
O(1) by construction) and applies the {0,1} mask multiplicatively
post-exp, which matches the reference's -1e9 masking exactly (both
give exact zeros).  Row normalization (1/rowsum) is applied to the
attention matrix before it is transposed (PE transposes) for the
A @ V chain, so no deferred scaling is needed downstream.
"""

import sys

try:
    import concourse.bass as bass
except ImportError:  # pragma: no cover - grading env fallback
    sys.path.insert(0, "/opt/trn_rl_repo")
    import concourse.bass as bass

import numpy as np

import concourse.bacc as bacc
import concourse.mybir as mybir
import concourse.tile as tile
from concourse.bass_utils import run_bass_kernel_spmd
from concourse.masks import make_identity

F32 = mybir.dt.float32
F32R = mybir.dt.float32r

B, N, D = 4, 1024, 1024
H, DK, DV = 8, 128, 128
TEMP = float(DK) ** 0.5
EPS = 1e-6
NR = N // 2  # rows per core (512)
NCORES = 8

AF = mybir.ActivationFunctionType
ALU = mybir.AluOpType


def _r(ap):
    return ap.bitcast(F32R)


def _load_rows(nc, pool, dram, ncols, tag, f32r=True):
    """Load a [1024, ncols] DRAM tensor as 8 [128, ncols] SBUF tiles.

    f32r=True tags the DMA output dtype float32r so walrus accepts the
    tiles as FP32r matmul operands.
    """
    ts_ = []
    for i in range(8):
        t = pool.tile([128, ncols], F32, tag=tag)
        src = dram[i * 128 : (i + 1) * 128, :]
        if f32r:
            nc.sync.dma_start(_r(t[:]), _r(src))
        else:
            nc.sync.dma_start(t[:], src)
        ts_.append(t)
    return ts_


def _phase_a(nc, psum, small, keep, xina, wina, qk2, xqfT, xkfT, wqfT, wkfT,
             af_out, Ftiles):
    """Feature attention: F[h] = softmax(qf_h.T @ kf_h / temp), [dv, dv2]."""
    xqf_t = _load_rows(nc, xina, xqfT, 1024, "xt")
    xkf_t = _load_rows(nc, xina, xkfT, 1024, "xt")
    wqf_t = _load_rows(nc, wina, wqfT, 1024, "wt")
    wkf_t = _load_rows(nc, wina, wkfT, 1024, "wt")
    for pair in range(4):
        qf2 = []
        kf2 = []
        for nt in range(8):
            pq = psum.tile([128, 256], F32, tag="ps")
            pk = psum.tile([128, 256], F32, tag="ps")
            for dt in range(8):
                nc.tensor.matmul(
                    pq[:],
                    _r(xqf_t[dt][:, nt * 128 : (nt + 1) * 128]),
                    _r(wqf_t[dt][:, pair * 256 : (pair + 1) * 256]),
                    start=(dt == 0),
                    stop=(dt == 7),
                )
            for dt in range(8):
                nc.tensor.matmul(
                    pk[:],
                    _r(xkf_t[dt][:, nt * 128 : (nt + 1) * 128]),
                    _r(wkf_t[dt][:, pair * 256 : (pair + 1) * 256]),
                    start=(dt == 0),
                    stop=(dt == 7),
                )
            tq = qk2.tile([128, 256], F32, tag=f"qf2_{nt}")
            tk = qk2.tile([128, 256], F32, tag=f"kf2_{nt}")
            nc.vector.tensor_copy(_r(tq[:]), pq[:])
            nc.vector.tensor_copy(_r(tk[:]), pk[:])
            qf2.append(tq)
            kf2.append(tk)
        for h2 in range(2):
            h = 2 * pair + h2
            pg = psum.tile([128, 128], F32, tag="ps")
            for nt in range(8):
                nc.tensor.matmul(
                    pg[:],
                    _r(qf2[nt][:, h2 * 128 : (h2 + 1) * 128]),
                    _r(kf2[nt][:, h2 * 128 : (h2 + 1) * 128]),
                    start=(nt == 0),
                    stop=(nt == 7),
                )
            eg = small.tile([128, 128], F32, tag="expG")
            nc.scalar.activation(eg[:], pg[:], AF.Exp, scale=1.0 / TEMP)
            gs = small.tile([128, 1], F32, tag="gsum")
            nc.vector.reduce_sum(gs[:], eg[:], axis=mybir.AxisListType.X)
            nc.vector.reciprocal(gs[:], gs[:])
            fh = keep.tile([128, 128], F32, tag=f"F{h}")
            nc.vector.tensor_scalar_mul(_r(fh[:]), eg[:], gs[:])
            nc.gpsimd.dma_start(af_out[h, :, :], fh[:])
            Ftiles.append(fh)


def _phase_b(nc, psum, vvp, xinb, winb, xvT, wvT, vv):
    """vv[mt] = (V @ Wv.T) tile  ([m-tile, h*dv])."""
    xv_t = _load_rows(nc, xinb, xvT, 1024, "xt")
    wv_t = _load_rows(nc, winb, wvT, 1024, "wt")
    for mt in range(8):
        t = vvp.tile([128, 1024], F32, tag=f"vv{mt}")
        for dh in range(2):
            ps = psum.tile([128, 512], F32, tag="ps")
            for dt in range(8):
                nc.tensor.matmul(
                    ps[:],
                    _r(xv_t[dt][:, mt * 128 : (mt + 1) * 128]),
                    _r(wv_t[dt][:, dh * 512 : (dh + 1) * 512]),
                    start=(dt == 0),
                    stop=(dt == 7),
                )
            nc.vector.tensor_copy(_r(t[:, dh * 512 : (dh + 1) * 512]), ps[:])
        vv.append(t)


def _phase_cq(nc, psum, qkp, xincq, wincq, xqT, wqtT, qtT):
    """qtT[h] = (Wqt_h @ XQ.T)  [dk, n]."""
    xq_t = _load_rows(nc, xincq, xqT, NR, "xtq")
    wqt_t = _load_rows(nc, wincq, wqtT, 1024, "wt")
    for h in range(H):
        tq = qkp.tile([128, NR], F32, tag=f"qtT{h}")
        ps = psum.tile([128, NR], F32, tag="ps")
        for dt in range(8):
            nc.tensor.matmul(
                ps[:],
                _r(wqt_t[dt][:, h * 128 : (h + 1) * 128]),
                _r(xq_t[dt][:]),
                start=(dt == 0),
                stop=(dt == 7),
            )
        nc.vector.tensor_copy(_r(tq[:]), ps[:])
        qtT.append(tq)


def _phase_ck(nc, psum, qkp, xinck, winck, xkT, wktT, ktT):
    """ktT[h] = (Wkt_h @ XK.T)  [dk, m]."""
    xk_t = _load_rows(nc, xinck, xkT, 1024, "xt")
    wkt_t = _load_rows(nc, winck, wktT, 1024, "wt")
    for h in range(H):
        tk = qkp.tile([128, 1024], F32, tag=f"ktT{h}")
        for mh in range(2):
            ps = psum.tile([128, 512], F32, tag="ps")
            for dt in range(8):
                nc.tensor.matmul(
                    ps[:],
                    _r(wkt_t[dt][:, h * 128 : (h + 1) * 128]),
                    _r(xk_t[dt][:, mh * 512 : (mh + 1) * 512]),
                    start=(dt == 0),
                    stop=(dt == 7),
                )
            nc.vector.tensor_copy(_r(tk[:, mh * 512 : (mh + 1) * 512]), ps[:])
        ktT.append(tk)


def _phase_d(nc, psum, small, maskp, work, atp, ztp, ident, maskf, at_out,
             qtT, ktT, vv, Ftiles, ZT):
    """Per-head: scores, masked softmax, attn_time out, (A@V)@F chain."""
    mask_t = []
    for nt in range(4):
        t = maskp.tile([128, 1024], F32, tag=f"mask{nt}")
        nc.sync.dma_start(t[:], maskf[nt * 128 : (nt + 1) * 128, :])
        mask_t.append(t)

    for h in range(H):
        sA = []
        for nt in range(4):
            a = work.tile([128, 1024], F32, tag=f"sA{nt}")
            for mh in range(2):
                ps = psum.tile([128, 512], F32, tag="ps")
                nc.tensor.matmul(
                    ps[:],
                    _r(qtT[h][:, nt * 128 : (nt + 1) * 128]),
                    _r(ktT[h][:, mh * 512 : (mh + 1) * 512]),
                    start=True,
                    stop=True,
                )
                nc.scalar.activation(
                    a[:, mh * 512 : (mh + 1) * 512],
                    ps[:],
                    AF.Exp,
                    scale=1.0 / TEMP,
                )
            rs = small.tile([128, 1], F32, tag=f"rs{nt}")
            nc.vector.scalar_tensor_tensor(
                out=a[:],
                in0=a[:],
                scalar=1.0,
                in1=mask_t[nt][:],
                op0=ALU.mult,
                op1=ALU.mult,
                accum_out=rs[:],
            )
            nc.vector.reciprocal(rs[:], rs[:])
            nc.vector.tensor_scalar_mul(a[:], a[:], rs[:])
            nc.gpsimd.dma_start(at_out[h, nt * 128 : (nt + 1) * 128, :], a[:])
            sA.append(a)
        # transpose A -> AT[mt] [m, n]
        AT = []
        for mt in range(8):
            t = atp.tile([128, NR], F32, tag=f"AT{mt}")
            for nt in range(4):
                pt = psum.tile([128, 128], F32, tag="ps")
                nc.tensor.transpose(
                    pt[:], sA[nt][:, mt * 128 : (mt + 1) * 128], ident[:]
                )
                nc.vector.tensor_copy(_r(t[:, nt * 128 : (nt + 1) * 128]), pt[:])
            AT.append(t)
        # U.T [dv, n] accumulated over m tiles
        pu = psum.tile([128, NR], F32, tag="ps")
        for mt in range(8):
            nc.tensor.matmul(
                pu[:],
                _r(vv[mt][:, h * 128 : (h + 1) * 128]),
                _r(AT[mt][:]),
                start=(mt == 0),
                stop=(mt == 7),
            )
        usb = work.tile([128, NR], F32, tag="usb")
        nc.vector.tensor_copy(_r(usb[:]), pu[:])
        # Z.T [dv2, n] = F.T @ U.T
        pz = psum.tile([128, NR], F32, tag="ps")
        nc.tensor.matmul(pz[:], _r(Ftiles[h][:]), _r(usb[:]), start=True, stop=True)
        zt = ztp.tile([128, NR], F32, tag=f"ZT{h}")
        nc.vector.tensor_copy(_r(zt[:]), pz[:])
        ZT.append(zt)


def _phase_e(nc, psum, small, keep, wine, rese, fcop, wfcT, residual, gamma,
             beta, y_out, ZT):
    """fc over all heads + residual + layernorm."""
    wfc_t = _load_rows(nc, wine, wfcT, 1024, "wt")
    res_t = []
    for nt in range(4):
        t = rese.tile([128, 1024], F32, tag="res")
        nc.sync.dma_start(t[:], residual[nt * 128 : (nt + 1) * 128, :])
        res_t.append(t)
    gb = keep.tile([128, D], F32, tag="gamma")
    bb = keep.tile([128, D], F32, tag="beta")
    nc.sync.dma_start(gb[:], gamma[0:1, :].to_broadcast((128, D)))
    nc.sync.dma_start(bb[:], beta[0:1, :].to_broadcast((128, D)))
    eps_t = keep.tile([128, 1], F32, tag="eps")
    nc.vector.memset(eps_t[:], EPS)

    for nt in range(4):
        fco = fcop.tile([128, D], F32, tag="fco")
        for dh in range(2):
            pf = psum.tile([128, 512], F32, tag="ps")
            for h2 in range(H):
                nc.tensor.matmul(
                    pf[:],
                    _r(ZT[h2][:, nt * 128 : (nt + 1) * 128]),
                    _r(wfc_t[h2][:, dh * 512 : (dh + 1) * 512]),
                    start=(h2 == 0),
                    stop=(h2 == 7),
                )
            nc.vector.tensor_tensor(
                fco[:, dh * 512 : (dh + 1) * 512],
                pf[:],
                res_t[nt][:, dh * 512 : (dh + 1) * 512],
                op=ALU.add,
            )
        stats = small.tile([128, 2, 6], F32, tag="bst")
        nc.vector.bn_stats(stats[:, 0, :], fco[:, 0:512])
        nc.vector.bn_stats(stats[:, 1, :], fco[:, 512:1024])
        mv = small.tile([128, 2], F32, tag="mv")
        nc.vector.bn_aggr(mv[:], stats[:])
        rstd = small.tile([128, 1], F32, tag="rstd")
        nc.scalar.activation(rstd[:], mv[:, 1:2], AF.Sqrt, bias=eps_t[:], scale=1.0)
        nc.vector.reciprocal(rstd[:], rstd[:])
        nc.vector.tensor_scalar(
            out=fco[:],
            in0=fco[:],
            scalar1=mv[:, 0:1],
            scalar2=rstd[:],
            op0=ALU.subtract,
            op1=ALU.mult,
        )
        nc.vector.tensor_tensor(fco[:], fco[:], gb[:], op=ALU.mult)
        nc.vector.tensor_tensor(fco[:], fco[:], bb[:], op=ALU.add)
        nc.gpsimd.dma_start(y_out[nt * 128 : (nt + 1) * 128, :], fco[:])


def _emit(nc):
    # ---- DRAM I/O ----
    xqT = nc.declare_dram_parameter("xqT", [D, NR], F32, isOutput=False)
    xkT = nc.declare_dram_parameter("xkT", [D, N], F32, isOutput=False)
    xqfT = nc.declare_dram_parameter("xqfT", [D, N], F32, isOutput=False)
    xkfT = nc.declare_dram_parameter("xkfT", [D, N], F32, isOutput=False)
    xvT = nc.declare_dram_parameter("xvT", [D, N], F32, isOutput=False)
    maskf = nc.declare_dram_parameter("maskf", [NR, N], F32, isOutput=False)
    residual = nc.declare_dram_parameter("residual", [NR, D], F32, isOutput=False)
    wqtT = nc.declare_dram_parameter("wqtT", [D, H * DK], F32, isOutput=False)
    wktT = nc.declare_dram_parameter("wktT", [D, H * DK], F32, isOutput=False)
    wqfT = nc.declare_dram_parameter("wqfT", [D, H * DV], F32, isOutput=False)
    wkfT = nc.declare_dram_parameter("wkfT", [D, H * DV], F32, isOutput=False)
    wvT = nc.declare_dram_parameter("wvT", [D, H * DV], F32, isOutput=False)
    wfcT = nc.declare_dram_parameter("wfcT", [H * DV, D], F32, isOutput=False)
    gamma = nc.declare_dram_parameter("gamma", [1, D], F32, isOutput=False)
    beta = nc.declare_dram_parameter("beta", [1, D], F32, isOutput=False)

    at_out = nc.declare_dram_parameter("at_out", [H, NR, N], F32, isOutput=True)
    af_out = nc.declare_dram_parameter("af_out", [H, DV, DV], F32, isOutput=True)
    y_out = nc.declare_dram_parameter("y_out", [NR, D], F32, isOutput=True)

    with tile.TileContext(nc) as tc:
        with (
            tc.tile_pool(name="keep", bufs=1) as keep,
            tc.tile_pool(name="small", bufs=4) as small,
            tc.tile_pool(name="psum", bufs=4, space="PSUM") as psum,
        ):
            ident = keep.tile([128, 128], F32, tag="ident")
            make_identity(nc, ident[:])

            Ftiles = []
            with (
                tc.tile_pool(name="xina", bufs=16) as xina,
                tc.tile_pool(name="wina", bufs=16) as wina,
                tc.tile_pool(name="qk2", bufs=2) as qk2,
            ):
                _phase_a(nc, psum, small, keep, xina, wina, qk2,
                         xqfT, xkfT, wqfT, wkfT, af_out, Ftiles)

            with tc.tile_pool(name="ztp", bufs=1) as ztp:
                ZT = []
                with tc.tile_pool(name="vvp", bufs=1) as vvp:
                    vv = []
                    with (
                        tc.tile_pool(name="xinb", bufs=8) as xinb,
                        tc.tile_pool(name="winb", bufs=8) as winb,
                    ):
                        _phase_b(nc, psum, vvp, xinb, winb, xvT, wvT, vv)
                    with tc.tile_pool(name="qkp", bufs=1) as qkp:
                        qtT = []
                        ktT = []
                        with (
                            tc.tile_pool(name="xincq", bufs=8) as xincq,
                            tc.tile_pool(name="wincq", bufs=8) as wincq,
                        ):
                            _phase_cq(nc, psum, qkp, xincq, wincq, xqT, wqtT, qtT)
                        with (
                            tc.tile_pool(name="xinck", bufs=8) as xinck,
                            tc.tile_pool(name="winck", bufs=8) as winck,
                        ):
                            _phase_ck(nc, psum, qkp, xinck, winck, xkT, wktT, ktT)
                        with (
                            tc.tile_pool(name="maskp", bufs=1) as maskp,
                            tc.tile_pool(name="work", bufs=2) as work,
                            tc.tile_pool(name="atp", bufs=1) as atp,
                        ):
                            _phase_d(nc, psum, small, maskp, work, atp, ztp,
                                     ident, maskf, at_out, qtT, ktT, vv,
                                     Ftiles, ZT)
                with (
                    tc.tile_pool(name="wine", bufs=8) as wine,
                    tc.tile_pool(name="rese", bufs=4) as rese,
                    tc.tile_pool(name="fcop", bufs=2) as fcop,
                ):
                    _phase_e(nc, psum, small, keep, wine, rese, fcop,
                             wfcT, residual, gamma, beta, y_out, ZT)

    return nc


_NC_CACHE = {}


def _get_nc():
    if "nc" not in _NC_CACHE:
        nc = bacc.Bacc()
        _emit(nc)
        nc.finalize()  # runs the Bacc legalization passes (wait splitting etc.)
        _NC_CACHE["nc"] = nc
    return _NC_CACHE["nc"]


def _prep_in_maps(q_time, k_time, q_feature, k_feature, v, attn_mask,
                  w_qs_time, w_ks_time, w_qs_feature, w_ks_feature, w_vs,
                  w_fc, ln_gamma, ln_beta):
    f = np.float32
    ct = lambda a: np.ascontiguousarray(a, dtype=f)
    shared = {
        "wqtT": ct(np.asarray(w_qs_time).T),
        "wktT": ct(np.asarray(w_ks_time).T),
        "wqfT": ct(np.asarray(w_qs_feature).T),
        "wkfT": ct(np.asarray(w_ks_feature).T),
        "wvT": ct(np.asarray(w_vs).T),
        "wfcT": ct(np.asarray(w_fc).T),
        "gamma": ct(np.asarray(ln_gamma).reshape(1, D)),
        "beta": ct(np.asarray(ln_beta).reshape(1, D)),
    }
    in_maps = []
    for c in range(NCORES):
        b, half = divmod(c, 2)
        rows = slice(half * NR, (half + 1) * NR)
        m = dict(shared)
        m["xqT"] = ct(np.asarray(q_time)[b, rows, :].T)
        m["xkT"] = ct(np.asarray(k_time)[b].T)
        m["xqfT"] = ct(np.asarray(q_feature)[b].T)
        m["xkfT"] = ct(np.asarray(k_feature)[b].T)
        m["xvT"] = ct(np.asarray(v)[b].T)
        m["maskf"] = ct(np.asarray(attn_mask)[b, rows, :])
        m["residual"] = ct(np.asarray(v)[b, rows, :])
        in_maps.append(m)
    return in_maps


def kernel(q_time, k_time, q_feature, k_feature, v, attn_mask,
           w_qs_time, w_ks_time, w_qs_feature, w_ks_feature, w_vs,
           w_fc, ln_gamma, ln_beta, _trace=False):
    nc = _get_nc()
    in_maps = _prep_in_maps(
        q_time, k_time, q_feature, k_feature, v, attn_mask,
        w_qs_time, w_ks_time, w_qs_feature, w_ks_feature, w_vs,
        w_fc, ln_gamma, ln_beta,
    )
    res = run_bass_kernel_spmd(
        nc, in_maps, core_ids=list(range(NCORES)), trace=_trace
    )
    out = np.empty((B, N, D), np.float32)
    attn_time = np.empty((B, H, N, N), np.float32)
    attn_feature = np.empty((B, H, DV, DV), np.float32)
    for c in range(NCORES):
        b, half = divmod(c, 2)
        rows = slice(half * NR, (half + 1) * NR)
        r = res.results[c]
        out[b, rows, :] = r["y_out"]
        attn_time[b, :, rows, :] = r["at_out"]
        if half == 0:
            attn_feature[b] = r["af_out"]
    kernel._last_results = res
    return out, attn_time, attn_feature


# revision 10
# speedup vs baseline: 1.0168x; 1.0168x over previous
"""MultiHeadCDSA Trainium2 kernel (8 NeuronCores, SPMD, no collectives).

Sharding: core c handles batch b = c//2 and row-half r = c%2 (512 of 1024
query rows).  K/V-side work (k/v/feature projections, attn_feature) is
computed redundantly by the two cores of a batch pair; q-side work
(scores, softmax, attn@V chain, fc, layernorm) is split by rows.

All matmul operands are laid out so the contraction dim sits on SBUF
partitions; the host pre-transposes activations/weights so the device
never transposes inputs.  Softmax skips max-subtraction (scores are
O(1) by construction) and applies the {0,1} mask multiplicatively
post-exp, which matches the reference's -1e9 masking exactly (both
give exact zeros).  Row normalization (1/rowsum) is applied to the
attention matrix before it is transposed (PE transposes) for the
A @ V chain, so no deferred scaling is needed downstream.
"""

import sys

try:
    import concourse.bass as bass
except ImportError:  # pragma: no cover - grading env fallback
    sys.path.insert(0, "/opt/trn_rl_repo")
    import concourse.bass as bass

import numpy as np

import concourse.bacc as bacc
import concourse.mybir as mybir
import concourse.tile as tile
from concourse.bass_utils import run_bass_kernel_spmd
from concourse.masks import make_identity

F32 = mybir.dt.float32
F32R = mybir.dt.float32r

B, N, D = 4, 1024, 1024
H, DK, DV = 8, 128, 128
TEMP = float(DK) ** 0.5
EPS = 1e-6
NR = N // 2  # rows per core (512)
NCORES = 8

AF = mybir.ActivationFunctionType
ALU = mybir.AluOpType


def _r(ap):
    return ap.bitcast(F32R)


def _load_rows(nc, pool, dram, ncols, tag, f32r=True):
    """Load a [1024, ncols] DRAM tensor as 8 [128, ncols] SBUF tiles.

    f32r=True tags the DMA output dtype float32r so walrus accepts the
    tiles as FP32r matmul operands.
    """
    ts_ = []
    for i in range(8):
        t = pool.tile([128, ncols], F32, tag=tag)
        src = dram[i * 128 : (i + 1) * 128, :]
        if f32r:
            nc.sync.dma_start(_r(t[:]), _r(src))
        else:
            nc.sync.dma_start(t[:], src)
        ts_.append(t)
    return ts_


def _load_interleaved(nc, specs):
    """Issue DMAs for several [1024, ncols] tensors interleaved by d-tile,
    so the first accumulation matmuls can start after ~one tile per input
    instead of after entire tensors."""
    outs = [[] for _ in specs]
    for i in range(8):
        for j, (pool, dram, ncols, tag) in enumerate(specs):
            t = pool.tile([128, ncols], F32, tag=tag)
            nc.sync.dma_start(_r(t[:]), _r(dram[i * 128 : (i + 1) * 128, :]))
            outs[j].append(t)
    return outs


def _phase_a(nc, psum, small, keep, xina, wina, qk2, xqfT, xkfT, wqfT, wkfT,
             af_out, Ftiles):
    """Feature attention: F[h] = softmax(qf_h.T @ kf_h / temp), [dv, dv2]."""
    xqf_t, wqf_t, xkf_t, wkf_t = _load_interleaved(nc, [
        (xina, xqfT, 1024, "xt"),
        (wina, wqfT, 1024, "wt"),
        (xina, xkfT, 1024, "xt"),
        (wina, wkfT, 1024, "wt"),
    ])
    for pair in range(4):
        qf2 = []
        kf2 = []
        for nt in range(8):
            pq = psum.tile([128, 256], F32, tag="ps")
            pk = psum.tile([128, 256], F32, tag="ps")
            for dt in range(8):
                nc.tensor.matmul(
                    pq[:],
                    _r(xqf_t[dt][:, nt * 128 : (nt + 1) * 128]),
                    _r(wqf_t[dt][:, pair * 256 : (pair + 1) * 256]),
                    start=(dt == 0),
                    stop=(dt == 7),
                )
            for dt in range(8):
                nc.tensor.matmul(
                    pk[:],
                    _r(xkf_t[dt][:, nt * 128 : (nt + 1) * 128]),
                    _r(wkf_t[dt][:, pair * 256 : (pair + 1) * 256]),
                    start=(dt == 0),
                    stop=(dt == 7),
                )
            tq = qk2.tile([128, 256], F32, tag=f"qf2_{nt}")
            tk = qk2.tile([128, 256], F32, tag=f"kf2_{nt}")
            nc.vector.tensor_copy(_r(tq[:]), pq[:])
            nc.vector.tensor_copy(_r(tk[:]), pk[:])
            qf2.append(tq)
            kf2.append(tk)
        for h2 in range(2):
            h = 2 * pair + h2
            pg = psum.tile([128, 128], F32, tag="ps")
            for nt in range(8):
                nc.tensor.matmul(
                    pg[:],
                    _r(qf2[nt][:, h2 * 128 : (h2 + 1) * 128]),
                    _r(kf2[nt][:, h2 * 128 : (h2 + 1) * 128]),
                    start=(nt == 0),
                    stop=(nt == 7),
                )
            eg = small.tile([128, 128], F32, tag="expG")
            nc.scalar.activation(eg[:], pg[:], AF.Exp, scale=1.0 / TEMP)
            gs = small.tile([128, 1], F32, tag="gsum")
            nc.vector.reduce_sum(gs[:], eg[:], axis=mybir.AxisListType.X)
            nc.vector.reciprocal(gs[:], gs[:])
            fh = keep.tile([128, 128], F32, tag=f"F{h}")
            nc.vector.tensor_scalar_mul(_r(fh[:]), eg[:], gs[:])
            nc.sync.dma_start(af_out[h, :, :], fh[:])
            Ftiles.append(fh)


def _phase_b(nc, psum, vvp, xinb, winb, xvT, wvT, vv):
    """vv[mt] = (V @ Wv.T) tile  ([m-tile, h*dv])."""
    xv_t, wv_t = _load_interleaved(nc, [
        (xinb, xvT, 1024, "xt"),
        (winb, wvT, 1024, "wt"),
    ])
    for mt in range(8):
        t = vvp.tile([128, 1024], F32, tag=f"vv{mt}")
        for dh in range(2):
            ps = psum.tile([128, 512], F32, tag="ps")
            for dt in range(8):
                nc.tensor.matmul(
                    ps[:],
                    _r(xv_t[dt][:, mt * 128 : (mt + 1) * 128]),
                    _r(wv_t[dt][:, dh * 512 : (dh + 1) * 512]),
                    start=(dt == 0),
                    stop=(dt == 7),
                )
            nc.vector.tensor_copy(_r(t[:, dh * 512 : (dh + 1) * 512]), ps[:])
        vv.append(t)


def _phase_cq(nc, psum, qkp, xincq, wincq, xqT, wqtT, qtT):
    """qtT[h] = (Wqt_h @ XQ.T)  [dk, n]."""
    xq_t, wqt_t = _load_interleaved(nc, [
        (xincq, xqT, NR, "xtq"),
        (wincq, wqtT, 1024, "wt"),
    ])
    for h in range(H):
        tq = qkp.tile([128, NR], F32, tag=f"qtT{h}")
        ps = psum.tile([128, NR], F32, tag="ps")
        for dt in range(8):
            nc.tensor.matmul(
                ps[:],
                _r(wqt_t[dt][:, h * 128 : (h + 1) * 128]),
                _r(xq_t[dt][:]),
                start=(dt == 0),
                stop=(dt == 7),
            )
        nc.vector.tensor_copy(_r(tq[:]), ps[:])
        qtT.append(tq)


def _phase_ck(nc, psum, qkp, xinck, winck, xkT, wktT, ktT):
    """ktT[h] = (Wkt_h @ XK.T)  [dk, m]."""
    xk_t, wkt_t = _load_interleaved(nc, [
        (xinck, xkT, 1024, "xt"),
        (winck, wktT, 1024, "wt"),
    ])
    for h in range(H):
        tk = qkp.tile([128, 1024], F32, tag=f"ktT{h}")
        for mh in range(2):
            ps = psum.tile([128, 512], F32, tag="ps")
            for dt in range(8):
                nc.tensor.matmul(
                    ps[:],
                    _r(wkt_t[dt][:, h * 128 : (h + 1) * 128]),
                    _r(xk_t[dt][:, mh * 512 : (mh + 1) * 512]),
                    start=(dt == 0),
                    stop=(dt == 7),
                )
            nc.vector.tensor_copy(_r(tk[:, mh * 512 : (mh + 1) * 512]), ps[:])
        ktT.append(tk)


def _phase_d(nc, psum, small, maskp, work, atp, ztp, ident, maskf, at_out,
             qtT, ktT, vv, Ftiles, ZT):
    """Per-head: scores, masked softmax, attn_time out, (A@V)@F chain."""
    mask_t = []
    for nt in range(4):
        t = maskp.tile([128, 1024], F32, tag=f"mask{nt}")
        nc.sync.dma_start(t[:], maskf[nt * 128 : (nt + 1) * 128, :])
        mask_t.append(t)

    for h in range(H):
        sA = []
        for nt in range(4):
            a = work.tile([128, 1024], F32, tag=f"sA{nt}")
            for mh in range(2):
                ps = psum.tile([128, 512], F32, tag="ps")
                nc.tensor.matmul(
                    ps[:],
                    _r(qtT[h][:, nt * 128 : (nt + 1) * 128]),
                    _r(ktT[h][:, mh * 512 : (mh + 1) * 512]),
                    start=True,
                    stop=True,
                )
                nc.scalar.activation(
                    a[:, mh * 512 : (mh + 1) * 512],
                    ps[:],
                    AF.Exp,
                    scale=1.0 / TEMP,
                )
            rs = small.tile([128, 1], F32, tag=f"rs{nt}")
            nc.vector.scalar_tensor_tensor(
                out=a[:],
                in0=a[:],
                scalar=1.0,
                in1=mask_t[nt][:],
                op0=ALU.mult,
                op1=ALU.mult,
                accum_out=rs[:],
            )
            nc.vector.reciprocal(rs[:], rs[:])
            nc.vector.tensor_scalar_mul(a[:], a[:], rs[:])
            nc.sync.dma_start(at_out[h, nt * 128 : (nt + 1) * 128, :], a[:])
            sA.append(a)
        # transpose A -> AT[mt] [m, n]: 4 transposes share one PSUM bank,
        # drained by a single DVE cast
        AT = []
        for mt in range(8):
            t = atp.tile([128, NR], F32, tag=f"AT{mt}")
            pt = psum.tile([128, NR], F32, tag="ps")
            for nt in range(4):
                nc.tensor.transpose(
                    pt[:, nt * 128 : (nt + 1) * 128],
                    sA[nt][:, mt * 128 : (mt + 1) * 128],
                    ident[:],
                )
            nc.vector.tensor_copy(_r(t[:]), pt[:])
            AT.append(t)
        # U.T [dv, n] accumulated over m tiles
        pu = psum.tile([128, NR], F32, tag="ps")
        for mt in range(8):
            nc.tensor.matmul(
                pu[:],
                _r(vv[mt][:, h * 128 : (h + 1) * 128]),
                _r(AT[mt][:]),
                start=(mt == 0),
                stop=(mt == 7),
            )
        usb = work.tile([128, NR], F32, tag="usb")
        nc.vector.tensor_copy(_r(usb[:]), pu[:])
        # Z.T [dv2, n] = F.T @ U.T
        pz = psum.tile([128, NR], F32, tag="ps")
        nc.tensor.matmul(pz[:], _r(Ftiles[h][:]), _r(usb[:]), start=True, stop=True)
        zt = ztp.tile([128, NR], F32, tag=f"ZT{h}")
        nc.vector.tensor_copy(_r(zt[:]), pz[:])
        ZT.append(zt)


def _phase_e(nc, psum, small, wine, rese, fcop, wfcT, residual, gamma,
             beta, y_out, ZT):
    """fc over all heads + residual + layernorm."""
    wfc_t = _load_rows(nc, wine, wfcT, 1024, "wt")
    res_t = []
    for nt in range(4):
        t = rese.tile([128, 1024], F32, tag="res")
        nc.sync.dma_start(t[:], residual[nt * 128 : (nt + 1) * 128, :])
        res_t.append(t)
    gb = rese.tile([128, D], F32, tag="gamma")
    bb = rese.tile([128, D], F32, tag="beta")
    nc.sync.dma_start(gb[:], gamma[0:1, :].to_broadcast((128, D)))
    nc.sync.dma_start(bb[:], beta[0:1, :].to_broadcast((128, D)))
    eps_t = rese.tile([128, 1], F32, tag="eps")
    nc.vector.memset(eps_t[:], EPS)

    for nt in range(4):
        fco = fcop.tile([128, D], F32, tag="fco")
        for dh in range(2):
            pf = psum.tile([128, 512], F32, tag="ps")
            for h2 in range(H):
                nc.tensor.matmul(
                    pf[:],
                    _r(ZT[h2][:, nt * 128 : (nt + 1) * 128]),
                    _r(wfc_t[h2][:, dh * 512 : (dh + 1) * 512]),
                    start=(h2 == 0),
                    stop=(h2 == 7),
                )
            nc.vector.tensor_tensor(
                fco[:, dh * 512 : (dh + 1) * 512],
                pf[:],
                res_t[nt][:, dh * 512 : (dh + 1) * 512],
                op=ALU.add,
            )
        stats = small.tile([128, 2, 6], F32, tag="bst")
        nc.vector.bn_stats(stats[:, 0, :], fco[:, 0:512])
        nc.vector.bn_stats(stats[:, 1, :], fco[:, 512:1024])
        mv = small.tile([128, 2], F32, tag="mv")
        nc.vector.bn_aggr(mv[:], stats[:])
        rstd = small.tile([128, 1], F32, tag="rstd")
        nc.scalar.activation(rstd[:], mv[:, 1:2], AF.Sqrt, bias=eps_t[:], scale=1.0)
        nc.vector.reciprocal(rstd[:], rstd[:])
        nc.vector.tensor_scalar(
            out=fco[:],
            in0=fco[:],
            scalar1=mv[:, 0:1],
            scalar2=rstd[:],
            op0=ALU.subtract,
            op1=ALU.mult,
        )
        nc.vector.tensor_tensor(fco[:], fco[:], gb[:], op=ALU.mult)
        nc.vector.tensor_tensor(fco[:], fco[:], bb[:], op=ALU.add)
        nc.sync.dma_start(y_out[nt * 128 : (nt + 1) * 128, :], fco[:])


def _emit(nc):
    # ---- DRAM I/O ----
    xqT = nc.declare_dram_parameter("xqT", [D, NR], F32, isOutput=False)
    xkT = nc.declare_dram_parameter("xkT", [D, N], F32, isOutput=False)
    xqfT = nc.declare_dram_parameter("xqfT", [D, N], F32, isOutput=False)
    xkfT = nc.declare_dram_parameter("xkfT", [D, N], F32, isOutput=False)
    xvT = nc.declare_dram_parameter("xvT", [D, N], F32, isOutput=False)
    maskf = nc.declare_dram_parameter("maskf", [NR, N], F32, isOutput=False)
    residual = nc.declare_dram_parameter("residual", [NR, D], F32, isOutput=False)
    wqtT = nc.declare_dram_parameter("wqtT", [D, H * DK], F32, isOutput=False)
    wktT = nc.declare_dram_parameter("wktT", [D, H * DK], F32, isOutput=False)
    wqfT = nc.declare_dram_parameter("wqfT", [D, H * DV], F32, isOutput=False)
    wkfT = nc.declare_dram_parameter("wkfT", [D, H * DV], F32, isOutput=False)
    wvT = nc.declare_dram_parameter("wvT", [D, H * DV], F32, isOutput=False)
    wfcT = nc.declare_dram_parameter("wfcT", [H * DV, D], F32, isOutput=False)
    gamma = nc.declare_dram_parameter("gamma", [1, D], F32, isOutput=False)
    beta = nc.declare_dram_parameter("beta", [1, D], F32, isOutput=False)

    at_out = nc.declare_dram_parameter("at_out", [H, NR, N], F32, isOutput=True)
    af_out = nc.declare_dram_parameter("af_out", [H, DV, DV], F32, isOutput=True)
    y_out = nc.declare_dram_parameter("y_out", [NR, D], F32, isOutput=True)

    with tile.TileContext(nc) as tc:
        with (
            tc.tile_pool(name="keep", bufs=1) as keep,
            tc.tile_pool(name="small", bufs=4) as small,
            tc.tile_pool(name="psum", bufs=4, space="PSUM") as psum,
        ):
            ident = keep.tile([128, 128], F32, tag="ident")
            make_identity(nc, ident[:])

            Ftiles = []
            with (
                tc.tile_pool(name="xina", bufs=16) as xina,
                tc.tile_pool(name="wina", bufs=16) as wina,
                tc.tile_pool(name="qk2", bufs=2) as qk2,
            ):
                _phase_a(nc, psum, small, keep, xina, wina, qk2,
                         xqfT, xkfT, wqfT, wkfT, af_out, Ftiles)

            with tc.tile_pool(name="ztp", bufs=1) as ztp:
                ZT = []
                with tc.tile_pool(name="vvp", bufs=1) as vvp:
                    vv = []
                    with (
                        tc.tile_pool(name="xinb", bufs=8) as xinb,
                        tc.tile_pool(name="winb", bufs=8) as winb,
                    ):
                        _phase_b(nc, psum, vvp, xinb, winb, xvT, wvT, vv)
                    with tc.tile_pool(name="qkp", bufs=1) as qkp:
                        qtT = []
                        ktT = []
                        with (
                            tc.tile_pool(name="xincq", bufs=8) as xincq,
                            tc.tile_pool(name="wincq", bufs=8) as wincq,
                        ):
                            _phase_cq(nc, psum, qkp, xincq, wincq, xqT, wqtT, qtT)
                        with (
                            tc.tile_pool(name="xinck", bufs=8) as xinck,
                            tc.tile_pool(name="winck", bufs=8) as winck,
                        ):
                            _phase_ck(nc, psum, qkp, xinck, winck, xkT, wktT, ktT)
                        with (
                            tc.tile_pool(name="maskp", bufs=1) as maskp,
                            tc.tile_pool(name="work", bufs=2) as work,
                            tc.tile_pool(name="atp", bufs=2) as atp,
                        ):
                            _phase_d(nc, psum, small, maskp, work, atp, ztp,
                                     ident, maskf, at_out, qtT, ktT, vv,
                                     Ftiles, ZT)
                with (
                    tc.tile_pool(name="wine", bufs=8) as wine,
                    tc.tile_pool(name="rese", bufs=4) as rese,
                    tc.tile_pool(name="fcop", bufs=2) as fcop,
                ):
                    _phase_e(nc, psum, small, wine, rese, fcop,
                             wfcT, residual, gamma, beta, y_out, ZT)

    return nc


_NC_CACHE = {}


def _get_nc():
    if "nc" not in _NC_CACHE:
        nc = bacc.Bacc()
        _emit(nc)
        nc.finalize()  # runs the Bacc legalization passes (wait splitting etc.)
        _NC_CACHE["nc"] = nc
    return _NC_CACHE["nc"]


def _prep_in_maps(q_time, k_time, q_feature, k_feature, v, attn_mask,
                  w_qs_time, w_ks_time, w_qs_feature, w_ks_feature, w_vs,
                  w_fc, ln_gamma, ln_beta):
    f = np.float32
    ct = lambda a: np.ascontiguousarray(a, dtype=f)
    shared = {
        "wqtT": ct(np.asarray(w_qs_time).T),
        "wktT": ct(np.asarray(w_ks_time).T),
        "wqfT": ct(np.asarray(w_qs_feature).T),
        "wkfT": ct(np.asarray(w_ks_feature).T),
        "wvT": ct(np.asarray(w_vs).T),
        "wfcT": ct(np.asarray(w_fc).T),
        "gamma": ct(np.asarray(ln_gamma).reshape(1, D)),
        "beta": ct(np.asarray(ln_beta).reshape(1, D)),
    }
    in_maps = []
    for c in range(NCORES):
        b, half = divmod(c, 2)
        rows = slice(half * NR, (half + 1) * NR)
        m = dict(shared)
        m["xqT"] = ct(np.asarray(q_time)[b, rows, :].T)
        m["xkT"] = ct(np.asarray(k_time)[b].T)
        m["xqfT"] = ct(np.asarray(q_feature)[b].T)
        m["xkfT"] = ct(np.asarray(k_feature)[b].T)
        m["xvT"] = ct(np.asarray(v)[b].T)
        m["maskf"] = ct(np.asarray(attn_mask)[b, rows, :])
        m["residual"] = ct(np.asarray(v)[b, rows, :])
        in_maps.append(m)
    return in_maps


def kernel(q_time, k_time, q_feature, k_feature, v, attn_mask,
           w_qs_time, w_ks_time, w_qs_feature, w_ks_feature, w_vs,
           w_fc, ln_gamma, ln_beta, _trace=False):
    nc = _get_nc()
    in_maps = _prep_in_maps(
        q_time, k_time, q_feature, k_feature, v, attn_mask,
        w_qs_time, w_ks_time, w_qs_feature, w_ks_feature, w_vs,
        w_fc, ln_gamma, ln_beta,
    )
    res = run_bass_kernel_spmd(
        nc, in_maps, core_ids=list(range(NCORES)), trace=_trace
    )
    out = np.empty((B, N, D), np.float32)
    attn_time = np.empty((B, H, N, N), np.float32)
    attn_feature = np.empty((B, H, DV, DV), np.float32)
    for c in range(NCORES):
        b, half = divmod(c, 2)
        rows = slice(half * NR, (half + 1) * NR)
        r = res.results[c]
        out[b, rows, :] = r["y_out"]
        attn_time[b, :, rows, :] = r["at_out"]
        if half == 0:
            attn_feature[b] = r["af_out"]
    kernel._last_results = res
    return out, attn_time, attn_feature
